# revision 2
# baseline (speedup 1.0000x reference)
"""BatchTopK SAE kernel for 8 Trainium2 NeuronCores.

Strategy (tensor-parallel over latent dim):
  - Each core owns a 2048-wide slice of the 16384 latent dim.
  - P1 encode: encoded_slice = x @ enc_w_slice.T + b  as fp16 3-pass split
    (hi/lo mantissa split of both operands -> fp32-grade accuracy at
    1 cycle/row on the PE), streamed batch-major, spilled to DRAM scratch.
  - P2 batch top-k: per latent column (partition row), exact 81st-largest
    over the 8192 batch via a max8/match_replace tournament; then one fused
    (y >= tau) * y pass produces the sparse slice.
  - P3 decode: decodedT_partial = dec_w_slice.T-contracted GEMM over the
    sparse slice (fp16 single pass).
  - Host: concat/transpose sparse slices, sum decoded partials + dec_b.
"""

import os
import sys

sys.path.insert(0, "/opt/trn_rl_repo")

import numpy as np

import concourse.bacc as bacc
import concourse.mybir as mybir
from concourse.tile import TileContext
from concourse.bass_utils import run_bass_kernel_spmd

F32 = mybir.dt.float32
F16 = mybir.dt.float16

INPUT_DIM = 768
LATENT_DIM = 16384
BATCH = int(os.environ.get("KB", "8192"))
NCORES = 8
SHARD = LATENT_DIM // NCORES  # 2048 latents per core
TOPK = max(1, int(BATCH * 0.01))

KD = INPUT_DIM // 128  # 6 contraction tiles
NB = SHARD // 128      # 16 latent blocks per core
ENC_CH = 2048          # encode batch chunk (4 PSUM banks)
NG1 = BATCH // ENC_CH  # encode n-groups
NCH = BATCH // 128     # tournament L1 chunks
DEC_CH = 512           # decode batch chunk (1 PSUM bank per m-tile)
NG3 = BATCH // DEC_CH
NEG = -3.0e38

_nc_cache = {}


def _build():
    nc = bacc.Bacc("TRN2", target_bir_lowering=False, debug=False,
                   num_devices=NCORES)
    xt1 = nc.dram_tensor("xt1", [INPUT_DIM, BATCH], F16, kind="ExternalInput")
    xt2 = nc.dram_tensor("xt2", [INPUT_DIM, BATCH], F16, kind="ExternalInput")
    wt1 = nc.dram_tensor("wt1", [INPUT_DIM, SHARD], F16, kind="ExternalInput")
    wt2 = nc.dram_tensor("wt2", [INPUT_DIM, SHARD], F16, kind="ExternalInput")
    encb = nc.dram_tensor("encb", [128, NB], F32, kind="ExternalInput")
    dec16 = nc.dram_tensor("dec16", [SHARD, INPUT_DIM], F16, kind="ExternalInput")
    sparse_t = nc.dram_tensor("sparse_t", [SHARD, BATCH], F32, kind="ExternalOutput")
    dect = nc.dram_tensor("dect", [INPUT_DIM, BATCH], F32, kind="ExternalOutput")
    kcnt = nc.dram_tensor("kcnt", [128, NB], F32, kind="ExternalOutput")
    encs = nc.dram_tensor("encs", [SHARD, BATCH], F32, kind="Internal")

    r_last = (TOPK - 1) // 8   # tournament L2 round holding rank TOPK
    s_last = (TOPK - 1) % 8    # slot within that round

    with TileContext(nc) as tc:
        # ---------------- P1: encode + spill ----------------
        with tc.tile_pool(name="wpool", bufs=1) as wpool, \
             tc.tile_pool(name="xpool", bufs=2) as xpool, \
             tc.tile_pool(name="ps1", bufs=2, space="PSUM") as ps1, \
             tc.tile_pool(name="st1", bufs=3) as st1:
            w1t, w2t = [], []
            for k in range(KD):
                t1 = wpool.tile([128, SHARD], F16, tag=f"w1_{k}")
                nc.sync.dma_start(t1[:, :], wt1[k * 128:(k + 1) * 128, :])
                w1t.append(t1)
                t2 = wpool.tile([128, SHARD], F16, tag=f"w2_{k}")
                nc.sync.dma_start(t2[:, :], wt2[k * 128:(k + 1) * 128, :])
                w2t.append(t2)
            encbt = wpool.tile([128, NB], F32, tag="encbt")
            nc.sync.dma_start(encbt[:, :], encb[:, :])

            for ng in range(NG1):
                cs = slice(ng * ENC_CH, (ng + 1) * ENC_CH)
                x1t, x2t = [], []
                for k in range(KD):
                    rs = slice(k * 128, (k + 1) * 128)
                    t1 = xpool.tile([128, ENC_CH], F16, tag=f"x1_{k}")
                    nc.sync.dma_start(t1[:, :], xt1[rs, cs])
                    x1t.append(t1)
                    t2 = xpool.tile([128, ENC_CH], F16, tag=f"x2_{k}")
                    nc.sync.dma_start(t2[:, :], xt2[rs, cs])
                    x2t.append(t2)
                for blk in range(NB):
                    bs = slice(blk * 128, (blk + 1) * 128)
                    ps = ps1.tile([128, ENC_CH], F32, tag="ps")
                    passes = ((w1t, x1t), (w2t, x1t), (w1t, x2t))
                    for k in range(KD):
                        for p, (wt, xt) in enumerate(passes):
                            first = (k == 0 and p == 0)
                            last = (k == KD - 1 and p == len(passes) - 1)
                            for nn in range(ENC_CH // 512):
                                ns = slice(nn * 512, (nn + 1) * 512)
                                nc.tensor.matmul(
                                    ps[:, ns], wt[k][:, bs], xt[k][:, ns],
                                    start=first, stop=last)
                    st = st1.tile([128, ENC_CH], F32, tag="st")
                    nc.vector.tensor_scalar_add(st[:, :], ps[:, :],
                                                encbt[:, blk:blk + 1])
                    nc.sync.dma_start(encs[bs, cs], st[:, :])

        # ---------------- P2: exact batch top-k + mask ----------------
        with tc.tile_pool(name="p2t", bufs=2) as p2t, \
             tc.tile_pool(name="p2sp", bufs=2) as p2sp, \
             tc.tile_pool(name="p2c", bufs=2) as p2c, \
             tc.tile_pool(name="p2s", bufs=2) as p2s, \
             tc.tile_pool(name="p2m", bufs=4) as p2m, \
             tc.tile_pool(name="p2cnt", bufs=1) as p2cnt:
            cntt = p2cnt.tile([128, NB], F32, tag="cntt")
            for blk in range(NB):
                bs = slice(blk * 128, (blk + 1) * 128)
                et = p2t.tile([128, BATCH], F32, tag="et")
                nc.sync.dma_start(et[:, :], encs[bs, :])
                cand = p2c.tile([128, NCH * 16], F32, tag="cand")
                for j in range(NCH):
                    ch = et[:, j * 128:(j + 1) * 128]
                    c0 = slice(j * 16, j * 16 + 8)
                    c1 = slice(j * 16 + 8, j * 16 + 16)
                    nc.vector.max(out=cand[:, c0], in_=ch)
                    scr = p2s.tile([128, 128], F32, tag="scr")
                    nc.vector.match_replace(out=scr[:, :], in_to_replace=cand[:, c0],
                                            in_values=ch, imm_value=NEG)
                    nc.vector.max(out=cand[:, c1], in_=scr[:, :])
                tau = None
                for r in range(r_last + 1):
                    t8 = p2m.tile([128, 8], F32, tag=f"t8_{r % 2}")
                    nc.vector.max(out=t8[:, :], in_=cand[:, :])
                    if r < r_last:
                        nc.vector.match_replace(out=cand[:, :], in_to_replace=t8[:, :],
                                                in_values=cand[:, :], imm_value=NEG)
                    tau = t8
                sp = p2sp.tile([128, BATCH], F32, tag="sp")
                nc.vector.scalar_tensor_tensor(
                    sp[:, :], et[:, :], tau[:, s_last:s_last + 1], et[:, :],
                    op0=mybir.AluOpType.is_ge, op1=mybir.AluOpType.mult,
                    accum_out=cntt[:, blk:blk + 1])
                nc.sync.dma_start(sparse_t[bs, :], sp[:, :])
            nc.sync.dma_start(kcnt[:, :], cntt[:, :])

        # ---------------- P3: decode ----------------
        with tc.tile_pool(name="dpool", bufs=1) as dpool, \
             tc.tile_pool(name="zpool", bufs=3) as zpool, \
             tc.tile_pool(name="z16p", bufs=3) as z16p, \
             tc.tile_pool(name="ps3", bufs=1, space="PSUM") as ps3, \
             tc.tile_pool(name="ost", bufs=3) as ostp:
            dect_t = []
            for blk in range(NB):
                t = dpool.tile([128, INPUT_DIM], F16, tag=f"d_{blk}")
                nc.sync.dma_start(t[:, :], dec16[blk * 128:(blk + 1) * 128, :])
                dect_t.append(t)
            for ng in range(NG3):
                cs = slice(ng * DEC_CH, (ng + 1) * DEC_CH)
                psm = [ps3.tile([128, DEC_CH], F32, tag=f"m{m}", name=f"psm{m}") for m in range(KD)]
                for blk in range(NB):
                    zt = zpool.tile([128, DEC_CH], F32, tag="zt")
                    nc.sync.dma_start(zt[:, :], sparse_t[blk * 128:(blk + 1) * 128, cs])
                    z16 = z16p.tile([128, DEC_CH], F16, tag="z16")
                    nc.vector.tensor_copy(z16[:, :], zt[:, :])
                    for m in range(KD):
                        nc.tensor.matmul(psm[m][:, :],
                                         dect_t[blk][:, m * 128:(m + 1) * 128],
                                         z16[:, :],
                                         start=(blk == 0), stop=(blk == NB - 1))
                for m in range(KD):
                    ot = ostp.tile([128, DEC_CH], F32, tag="ot")
                    nc.scalar.copy(ot[:, :], psm[m][:, :])
                    nc.sync.dma_start(dect[m * 128:(m + 1) * 128, cs], ot[:, :])
    nc.compile()
    return nc


def _get_nc():
    if "nc" not in _nc_cache:
        _nc_cache["nc"] = _build()
    return _nc_cache["nc"]


def _split16(a):
    hi = a.astype(np.float16)
    lo = (a - hi.astype(np.float32)).astype(np.float16)
    return np.ascontiguousarray(hi), np.ascontiguousarray(lo)


def kernel(x, enc_w, enc_b, dec_w, dec_b):
    x = np.asarray(x, dtype=np.float32)
    enc_w = np.asarray(enc_w, dtype=np.float32)
    enc_b = np.asarray(enc_b, dtype=np.float32)
    dec_w = np.asarray(dec_w, dtype=np.float32)
    dec_b = np.asarray(dec_b, dtype=np.float32)

    xt = np.ascontiguousarray(x.T)  # [768, B]
    xt1, xt2 = _split16(xt)
    dec_wt = np.ascontiguousarray(dec_w.T)  # [16384, 768]

    in_maps = []
    for c in range(NCORES):
        ls = slice(c * SHARD, (c + 1) * SHARD)
        wt = np.ascontiguousarray(enc_w[ls, :].T)  # [768, SHARD]
        wt1, wt2 = _split16(wt)
        encb_r = np.ascontiguousarray(enc_b[ls].reshape(NB, 128).T)  # [128, NB]
        d16 = np.ascontiguousarray(dec_wt[ls, :]).astype(np.float16)
        in_maps.append({"xt1": xt1, "xt2": xt2, "wt1": wt1, "wt2": wt2,
                        "encb": encb_r, "dec16": d16})

    nc = _get_nc()
    trace = bool(int(os.environ.get("KERNEL_TRACE", "0")))
    res = run_bass_kernel_spmd(nc, in_maps, core_ids=list(range(NCORES)),
                               trace=trace)
    if trace:
        kernel.last_exec_time_ns = res.exec_time_ns
        kernel.last_trace = (res.instructions_and_trace[1]
                             if res.instructions_and_trace else None)

    sparse = np.empty((BATCH, LATENT_DIM), dtype=np.float32)
    decT = np.zeros((INPUT_DIM, BATCH), dtype=np.float64)
    kcounts = []
    for c in range(NCORES):
        r = res.results[c]
        sparse[:, c * SHARD:(c + 1) * SHARD] = r["sparse_t"].T
        decT += r["dect"]
        kcounts.append(r["kcnt"].T.reshape(-1))
    decoded = (decT.T + dec_b.astype(np.float64)).astype(np.float32)
    kernel.last_kcounts = np.concatenate(kcounts)
    return decoded, sparse
